# revision 24
# baseline (speedup 1.0000x reference)
"""Causal depthwise conv (B=8, L=4096, D=1024, K=15) on 8 TRN2 NeuronCores.

Sharding: channels split across the 8 cores (128 channels each); every core
processes all 8 batch sequences for its channel slice. Host re-lays-out x to
[channels, batch, time] fp16 so on-chip tiles have channels on SBUF
partitions and time on the free dimension; tap shifts are free-dim offsets.

Engine split of the 15 taps (fp16 compute, fp32 PSUM accumulation):
  - TensorE (9 taps {0,1,3,5,7,9,11,13,14}): diagonal-weight matmuls into
    two 2048-wide PSUM halves, tap-outer / q-inner so each diag weight
    stays loaded for 4 consecutive matmuls. For batches 1-7 ALL matmuls
    run start=False: each PSUM half is pre-initialized by the ScalarE
    product of tap 8, a free fold (explicit add_dep_helper edges root
    every bank's write chain in the init - the scheduler treats matmuls
    as pure writers and would otherwise hoist them above it; one
    semaphore on the first matmul, nosync ordering edges for the other
    banks). Batch 0 uses plain start=True groups: on the very first
    execution after NEFF load, act-init products written during the
    cold-start window were observed to vanish (cores 1-7, batch 0 only),
    so batch 0 instead gives tap 8 to the DVE.
  - ScalarE: tap-8 init products, tap 12 (4096-wide), tap 10's second
    half, and the PSUM->SBUF fp16 bridge after the PE finishes each
    half. The tap-12 product is emitted between the two half-groups so
    the in-order ScalarE queue never head-of-line blocks on the PE.
  - DVE (taps {2,4,6} + tap 10's first half): tensor_scalar muls in 4x
    packed mode (even offsets keep 4B alignment), a fold chain of
    4096-wide tensor_tensor adds, and a single 4096-wide merge with the
    bridged PE partial.
Software pipelined: iteration b runs PE(b) while bridging/merging/storing
batch b-1; PSUM is recycled at half (4-bank) granularity so
bridge(b-1,h) -> init(b,h) hides under the other half's matmuls and the
PE never stalls (steady state 15.87us/batch = 220ns/matmul, zero gaps).
Ten dependency-free warm-up matmuls during the startup DMAs ramp the PE
power-state so batch 0 also runs at full speed. The final batch runs a
512-chunked epilogue whose last chunks merge directly from PSUM. Output
is fp16; the host upcasts to fp32 (rel err ~4.6e-4 vs fp32 reference).
"""

from contextlib import ExitStack

import numpy as np

import concourse.bacc as bacc
import concourse.tile as tile
from concourse import mybir
from concourse.bass_utils import run_bass_kernel_spmd
from concourse.tile import add_dep_helper

F32 = mybir.dt.float32
F16 = mybir.dt.float16
F16NP = np.float16

B = 8
L = 4096
D = 1024
K = 15
NCORES = 8
CPC = D // NCORES  # channels per core = 128
LP = L + K - 1  # 4110

PE_TAPS = [0, 1, 3, 5, 7, 9, 11, 13, 14]
INIT_TAP = 8          # ScalarE product written straight into PSUM
SC_TAPS = [10, 12]    # ScalarE products to SBUF
DVE_TAPS = [2, 4, 6]  # DVE 4x-mode products (even offsets)

_compiled_nc = None
_last_in_maps = None


def _build_nc():
    nc = bacc.Bacc(
        "TRN2",
        target_bir_lowering=False,
        debug=False,
        enable_asserts=True,
        num_devices=NCORES,
    )
    x = nc.dram_tensor("x", [CPC, B, LP], F16, kind="ExternalInput").ap()
    diag = nc.dram_tensor("diag", [CPC, len(PE_TAPS) * CPC], F16, kind="ExternalInput").ap()
    w = nc.dram_tensor("w", [CPC, 16], F32, kind="ExternalInput").ap()
    out = nc.dram_tensor("out", [CPC, B, L], F16, kind="ExternalOutput").ap()

    add = mybir.AluOpType.add

    with tile.TileContext(nc) as tc, ExitStack() as ctx:
        const_pool = ctx.enter_context(tc.tile_pool(name="const", bufs=1))
        xp = ctx.enter_context(tc.tile_pool(name="xp", bufs=4))
        prodp = ctx.enter_context(tc.tile_pool(name="prodp", bufs=7))
        sump = ctx.enter_context(tc.tile_pool(name="sump", bufs=6))
        accp = ctx.enter_context(tc.tile_pool(name="accp", bufs=2))
        op = ctx.enter_context(tc.tile_pool(name="op", bufs=2))
        pp = ctx.enter_context(tc.tile_pool(name="pp", bufs=1, space="PSUM"))

        # Startup: diag on the scalar queue in parallel with x(b0) pieces on
        # sync; b0-h0 runs q-outer so the first matmuls need only the first
        # 528 columns.
        dg = const_pool.tile([CPC, len(PE_TAPS) * CPC], F16, tag="diag")
        nc.scalar.dma_start(dg[:], diag[:])
        wt = const_pool.tile([CPC, 16], F32, tag="w")
        nc.sync.dma_start(wt[:], w[:])
        xt0 = xp.tile([CPC, LP], F16, tag="x", name="x_0")
        for s0, s1 in [(0, 528), (528, 1056), (1056, 1584), (1584, 2080)]:
            nc.sync.dma_start(xt0[:, s0:s1], x[:, 0, s0:s1])
        nc.scalar.dma_start(xt0[:, 2080:LP], x[:, 0, 2080:LP])

        # PE warm-up: dependency-free dummy matmuls during the startup DMAs
        # ramp the PE out of its low power-state before batch 0's real work.
        # They write the ps0 PSUM slot, which batch 0 re-opens with
        # start=True, so the garbage never escapes.
        scr = const_pool.tile([CPC, 512], F16, tag="scratch")
        nc.gpsimd.memset(scr[:], 0.0)
        ps_warm = pp.tile([CPC, 512], F32, tag="ps0", name="ps_warm")
        for _ in range(8):
            nc.tensor.matmul(ps_warm[:], scr[:, 0:CPC], scr[:],
                             start=True, stop=True, skip_group_check=True)

        xts = {0: xt0}
        state = {}  # carries batch b-1's tiles into iteration b

        def bridge_half(b, h, ps):
            """ScalarE PSUM->SBUF fp16 for half h of batch b (prev batch)."""
            acc = state[b]["acc"]
            nc.scalar.copy(acc[:, h * 2048 : (h + 1) * 2048], ps[:])

        def merge_and_store(b):
            """DVE merges + store for batch b (prev batch)."""
            acc, s = state[b]["acc"], state[b]["s"]
            ot = op.tile([CPC, L], F16, tag="osb", name=f"o_{b}")
            nc.vector.tensor_tensor(ot[:], s[:], acc[:], add)
            nc.sync.dma_start(out[:, b, :], ot[:])

        for b in range(B):
            xt = xts[b]
            # prefetch x(b+1) in one DMA
            if b + 1 < B:
                xn = xp.tile([CPC, LP], F16, tag="x", name=f"x_{b + 1}")
                nc.sync.dma_start(xn[:], x[:, b + 1, :])
                xts[b + 1] = xn

            # Batch 0 runs the proven start=True scheme (its PSUM activity
            # falls in the cold-start window where act-init writes were
            # observed to be lost on first execution); later batches use the
            # act-init with an explicit read-back drain guard.
            use_init = b > 0

            pss = []
            for h in range(2):
                t0 = h * 2048
                # bridge previous batch's half (frees the PSUM slot) ...
                if b > 0:
                    bridge_half(b - 1, h, state[b - 1]["ps"][h])
                # ... then init this batch's half and run the PE taps.
                ps = pp.tile([CPC, 2048], F32, tag=f"ps{h}", name=f"ps_{b}_{h}")
                guard = None
                if use_init:
                    guard = nc.scalar.mul(
                        ps[:], xt[:, t0 + INIT_TAP : t0 + INIT_TAP + 2048],
                        wt[:, INIT_TAP : INIT_TAP + 1])
                if b == 0 and h == 0:
                    # q-outer: match the PE start to the x DMA arrival pace
                    loop = [(ji, k, q) for q in range(4)
                            for ji, k in enumerate(PE_TAPS)]
                else:
                    loop = [(ji, k, q) for ji, k in enumerate(PE_TAPS)
                            for q in range(4)]
                root_mm = None
                for ji, k, q in loop:
                    last_tap = ji == len(PE_TAPS) - 1
                    mm = nc.tensor.matmul(
                        ps[:, q * 512 : (q + 1) * 512],
                        dg[:, ji * CPC : (ji + 1) * CPC],
                        xt[:, t0 + k + q * 512 : t0 + k + (q + 1) * 512],
                        start=(not use_init) and ji == 0,
                        stop=last_tap,
                        skip_group_check=True,
                    )
                    if use_init and ji == 0:
                        # The scheduler sees matmuls as pure writers, so each
                        # bank's WAW chain must be rooted in the (drained)
                        # act-init. Only the first matmul takes a semaphore;
                        # the other banks' roots order behind it for free
                        # (PE executes in order).
                        if root_mm is None:
                            root_mm = mm
                            add_dep_helper(
                                mm.ins, guard.ins, sync=True,
                                reason="PSUM must be act-initialized and "
                                       "drained before accumulating",
                            )
                        else:
                            add_dep_helper(
                                mm.ins, root_mm.ins, sync=False,
                                reason="bank root ordered behind guarded "
                                       "root matmul",
                            )
                pss.append(ps)

                if h == 0:
                    # Emit ScalarE's tap-12 product between the half groups:
                    # it fills ScalarE's slot while waiting for the PE to
                    # finish h1 of the previous batch (avoids head-of-line
                    # blocking on the in-order ScalarE queue).
                    prods = {}
                    pt12 = prodp.tile([CPC, L], F16, tag="prod", name=f"sp_{b}_12")
                    nc.scalar.mul(pt12[:], xt[:, 12 : 12 + L], wt[:, 12:13])
                    prods[12] = pt12

            # Tap 10: second half on ScalarE, first half on DVE (keeps
            # ScalarE under the PE's per-batch budget).
            pt10 = prodp.tile([CPC, L], F16, tag="prod", name=f"sp_{b}_10")
            nc.scalar.mul(pt10[:, 2048:L], xt[:, 2048 + 10 : 2048 + 10 + 2048],
                          wt[:, 10:11])

            # previous batch: DVE merges + store. For the last batch, emit
            # this AFTER the fold chain: the epilogue is gated by s(B-1), so
            # the chain must lead the DVE queue there.
            if b > 0 and b < B - 1:
                merge_and_store(b - 1)

            # DVE products + fold chain for this batch (b0 also owns tap 8)
            dve_taps = DVE_TAPS if use_init else DVE_TAPS + [INIT_TAP]
            for k in dve_taps:
                pt = prodp.tile([CPC, L], F16, tag="prod", name=f"dp_{b}_{k}")
                nc.vector.tensor_scalar_mul(pt[:], xt[:, k : k + L], wt[:, k : k + 1])
                prods[k] = pt
            nc.vector.tensor_scalar_mul(pt10[:, 0:2048], xt[:, 10 : 10 + 2048],
                                        wt[:, 10:11])
            prods[10] = pt10
            s = prods[2]
            for i, k in enumerate(dve_taps[1:] + [12, 10]):
                dst = sump.tile([CPC, L], F16, tag="sum", name=f"s_{b}_{i}")
                nc.vector.tensor_tensor(dst[:], prods[k][:], s[:], add)
                s = dst
            if b == B - 1:
                merge_and_store(b - 1)

            acc = accp.tile([CPC, L], F16, tag="acc", name=f"acc_{b}")
            state[b] = {"ps": pss, "s": s, "acc": acc}

        # Epilogue for the last batch: 1024-chunked bridge/merge/store.
        b = B - 1
        acc, s, pss = state[b]["acc"], state[b]["s"], state[b]["ps"]
        # Epilogue stores alternate between the sync and scalar DMA queues:
        # a single queue's ~0.65us issue rate would serialize the tail.
        store_engines = [nc.sync, nc.scalar]
        nst = 0
        ot = op.tile([CPC, L], F16, tag="osb", name=f"o_{b}")
        for c in range(2):
            sl = slice(c * 1024, (c + 1) * 1024)
            nc.scalar.copy(acc[:, sl], pss[0][:, sl])
            nc.vector.tensor_tensor(ot[:, sl], s[:, sl], acc[:, sl], add)
            store_engines[nst % 2].dma_start(out[:, b, sl], ot[:, sl])
            nst += 1
        for c in range(2):
            sl = slice(2048 + c * 512, 2048 + (c + 1) * 512)
            psl = slice(c * 512, (c + 1) * 512)
            nc.scalar.copy(acc[:, sl], pss[1][:, psl])
            nc.vector.tensor_tensor(ot[:, sl], s[:, sl], acc[:, sl], add)
            store_engines[nst % 2].dma_start(out[:, b, sl], ot[:, sl])
            nst += 1
        # Last two chunks: DVE merges straight from PSUM (skips the bridge
        # hop on the serial tail).
        for c in range(2, 4):
            sl = slice(2048 + c * 512, 2048 + (c + 1) * 512)
            psl = slice(c * 512, (c + 1) * 512)
            nc.vector.tensor_tensor(ot[:, sl], pss[1][:, psl], s[:, sl], add)
            store_engines[nst % 2].dma_start(out[:, b, sl], ot[:, sl])
            nst += 1

    nc.compile()
    return nc


def kernel(x: np.ndarray, weight: np.ndarray) -> np.ndarray:
    """x: [8, 4096, 1024] fp32, weight: [15, 1, 1024] fp32 ->
    [8, 4096, 1024] fp32 causal depthwise conv."""
    global _compiled_nc
    if _compiled_nc is None:
        _compiled_nc = _build_nc()
    nc = _compiled_nc

    x = np.ascontiguousarray(x, dtype=np.float32)
    wk = np.ascontiguousarray(weight, dtype=np.float32).reshape(K, D)
    x16 = x.astype(F16NP)
    wk16 = wk.astype(F16NP)

    in_maps = []
    for c in range(NCORES):
        sl = slice(c * CPC, (c + 1) * CPC)
        xpad = np.zeros((CPC, B, LP), dtype=F16NP)
        xpad[:, :, K - 1 :] = x16[:, :, sl].transpose(2, 0, 1)
        dgc = np.zeros((CPC, len(PE_TAPS) * CPC), dtype=F16NP)
        didx = np.arange(CPC)
        for j, k in enumerate(PE_TAPS):
            dgc[didx, j * CPC + didx] = wk16[k, sl]
        wt = np.zeros((CPC, 16), dtype=np.float32)
        wt[:, :K] = wk[:, sl].T
        in_maps.append({"x": xpad, "diag": dgc, "w": wt})

    global _last_in_maps
    _last_in_maps = in_maps
    res = run_bass_kernel_spmd(nc, in_maps, list(range(NCORES)))

    out = np.empty((B, L, D), dtype=np.float32)
    for c in range(NCORES):
        sl = slice(c * CPC, (c + 1) * CPC)
        out[:, :, sl] = res.results[c]["out"].transpose(1, 2, 0).astype(np.float32)
    return out


# revision 25
# speedup vs baseline: 1.0072x; 1.0072x over previous
"""Causal depthwise conv (B=8, L=4096, D=1024, K=15) on 8 TRN2 NeuronCores.

Sharding: channels split across the 8 cores (128 channels each); every core
processes all 8 batch sequences for its channel slice. Host re-lays-out x to
[channels, batch, time] fp16 so on-chip tiles have channels on SBUF
partitions and time on the free dimension; tap shifts are free-dim offsets.

Engine split of the 15 taps (fp16 compute, fp32 PSUM accumulation):
  - TensorE (9 taps {0,1,3,5,7,9,11,13,14}): diagonal-weight matmuls into
    two 2048-wide PSUM halves, tap-outer / q-inner so each diag weight
    stays loaded for 4 consecutive matmuls. For batches 1-7 ALL matmuls
    run start=False: each PSUM half is pre-initialized by the ScalarE
    product of tap 8, a free fold (explicit add_dep_helper edges root
    every bank's write chain in the init - the scheduler treats matmuls
    as pure writers and would otherwise hoist them above it; one
    semaphore on the first matmul, nosync ordering edges for the other
    banks). Batch 0 uses plain start=True groups: on the very first
    execution after NEFF load, act-init products written during the
    cold-start window were observed to vanish (cores 1-7, batch 0 only),
    so batch 0 instead gives tap 8 to the DVE.
  - ScalarE: tap-8 init products, tap 12 (4096-wide), tap 10's second
    half, and the PSUM->SBUF fp16 bridge after the PE finishes each
    half. The tap-12 product is emitted between the two half-groups so
    the in-order ScalarE queue never head-of-line blocks on the PE.
  - DVE (taps {2,4,6} + tap 10's first half): tensor_scalar muls in 4x
    packed mode (even offsets keep 4B alignment), a fold chain of
    4096-wide tensor_tensor adds, and a single 4096-wide merge with the
    bridged PE partial.
Software pipelined: iteration b runs PE(b) while bridging/merging/storing
batch b-1; PSUM is recycled at half (4-bank) granularity so
bridge(b-1,h) -> init(b,h) hides under the other half's matmuls and the
PE never stalls (steady state 15.87us/batch = 220ns/matmul, zero gaps).
Ten dependency-free warm-up matmuls during the startup DMAs ramp the PE
power-state so batch 0 also runs at full speed. The final batch runs a
512-chunked epilogue whose last chunks merge directly from PSUM. Output
is fp16; the host upcasts to fp32 (rel err ~4.6e-4 vs fp32 reference).
"""

from contextlib import ExitStack

import numpy as np

import concourse.bacc as bacc
import concourse.tile as tile
from concourse import mybir
from concourse.bass_utils import run_bass_kernel_spmd
from concourse.tile import add_dep_helper

F32 = mybir.dt.float32
F16 = mybir.dt.float16
F16NP = np.float16

B = 8
L = 4096
D = 1024
K = 15
NCORES = 8
CPC = D // NCORES  # channels per core = 128
LP = L + K - 1  # 4110

PE_TAPS = [0, 1, 3, 5, 7, 9, 11, 13, 14]
INIT_TAP = 8          # ScalarE product written straight into PSUM
SC_TAPS = [10, 12]    # ScalarE products to SBUF
DVE_TAPS = [2, 4, 6]  # DVE 4x-mode products (even offsets)

_compiled_nc = None
_last_in_maps = None


def _build_nc():
    nc = bacc.Bacc(
        "TRN2",
        target_bir_lowering=False,
        debug=False,
        enable_asserts=True,
        num_devices=NCORES,
    )
    x = nc.dram_tensor("x", [CPC, B, LP], F16, kind="ExternalInput").ap()
    diag = nc.dram_tensor("diag", [CPC, len(PE_TAPS) * CPC], F16, kind="ExternalInput").ap()
    w = nc.dram_tensor("w", [CPC, 16], F32, kind="ExternalInput").ap()
    out = nc.dram_tensor("out", [CPC, B, L], F16, kind="ExternalOutput").ap()

    add = mybir.AluOpType.add

    with tile.TileContext(nc) as tc, ExitStack() as ctx:
        const_pool = ctx.enter_context(tc.tile_pool(name="const", bufs=1))
        xp = ctx.enter_context(tc.tile_pool(name="xp", bufs=4))
        prodp = ctx.enter_context(tc.tile_pool(name="prodp", bufs=7))
        sump = ctx.enter_context(tc.tile_pool(name="sump", bufs=6))
        accp = ctx.enter_context(tc.tile_pool(name="accp", bufs=2))
        op = ctx.enter_context(tc.tile_pool(name="op", bufs=2))
        pp = ctx.enter_context(tc.tile_pool(name="pp", bufs=1, space="PSUM"))

        # Startup: diag on the scalar queue in parallel with x(b0) pieces on
        # sync; b0-h0 runs q-outer so the first matmuls need only the first
        # 528 columns.
        dg = const_pool.tile([CPC, len(PE_TAPS) * CPC], F16, tag="diag")
        nc.scalar.dma_start(dg[:], diag[:])
        wt = const_pool.tile([CPC, 16], F32, tag="w")
        nc.sync.dma_start(wt[:], w[:])
        xt0 = xp.tile([CPC, LP], F16, tag="x", name="x_0")
        for s0, s1 in [(0, 528), (528, 1056), (1056, 1584), (1584, 2080)]:
            nc.sync.dma_start(xt0[:, s0:s1], x[:, 0, s0:s1])
        nc.scalar.dma_start(xt0[:, 2080:LP], x[:, 0, 2080:LP])

        # PE warm-up: dependency-free dummy matmuls during the startup DMAs
        # ramp the PE out of its low power-state before batch 0's real work.
        # They write the ps0 PSUM slot, which batch 0 re-opens with
        # start=True, so the garbage never escapes.
        scr = const_pool.tile([CPC, 512], F16, tag="scratch")
        nc.gpsimd.memset(scr[:], 0.0)
        ps_warm = pp.tile([CPC, 512], F32, tag="ps0", name="ps_warm")
        for _ in range(6):
            nc.tensor.matmul(ps_warm[:], scr[:, 0:CPC], scr[:],
                             start=True, stop=True, skip_group_check=True)

        xts = {0: xt0}
        state = {}  # carries batch b-1's tiles into iteration b

        def bridge_half(b, h, ps):
            """ScalarE PSUM->SBUF fp16 for half h of batch b (prev batch)."""
            acc = state[b]["acc"]
            nc.scalar.copy(acc[:, h * 2048 : (h + 1) * 2048], ps[:])

        def merge_and_store(b):
            """DVE merges + store for batch b (prev batch)."""
            acc, s = state[b]["acc"], state[b]["s"]
            ot = op.tile([CPC, L], F16, tag="osb", name=f"o_{b}")
            nc.vector.tensor_tensor(ot[:], s[:], acc[:], add)
            nc.sync.dma_start(out[:, b, :], ot[:])

        for b in range(B):
            xt = xts[b]
            # prefetch x(b+1) in one DMA
            if b + 1 < B:
                xn = xp.tile([CPC, LP], F16, tag="x", name=f"x_{b + 1}")
                nc.sync.dma_start(xn[:], x[:, b + 1, :])
                xts[b + 1] = xn

            # Batch 0 runs the proven start=True scheme (its PSUM activity
            # falls in the cold-start window where act-init writes were
            # observed to be lost on first execution); later batches use the
            # act-init with an explicit read-back drain guard.
            use_init = b > 0

            pss = []
            for h in range(2):
                t0 = h * 2048
                # bridge previous batch's half (frees the PSUM slot) ...
                if b > 0:
                    bridge_half(b - 1, h, state[b - 1]["ps"][h])
                # ... then init this batch's half and run the PE taps.
                ps = pp.tile([CPC, 2048], F32, tag=f"ps{h}", name=f"ps_{b}_{h}")
                guard = None
                if use_init:
                    guard = nc.scalar.mul(
                        ps[:], xt[:, t0 + INIT_TAP : t0 + INIT_TAP + 2048],
                        wt[:, INIT_TAP : INIT_TAP + 1])
                if b == 0 and h == 0:
                    # q-outer: match the PE start to the x DMA arrival pace
                    loop = [(ji, k, q) for q in range(4)
                            for ji, k in enumerate(PE_TAPS)]
                else:
                    loop = [(ji, k, q) for ji, k in enumerate(PE_TAPS)
                            for q in range(4)]
                root_mm = None
                for ji, k, q in loop:
                    last_tap = ji == len(PE_TAPS) - 1
                    mm = nc.tensor.matmul(
                        ps[:, q * 512 : (q + 1) * 512],
                        dg[:, ji * CPC : (ji + 1) * CPC],
                        xt[:, t0 + k + q * 512 : t0 + k + (q + 1) * 512],
                        start=(not use_init) and ji == 0,
                        stop=last_tap,
                        skip_group_check=True,
                    )
                    if use_init and ji == 0:
                        # The scheduler sees matmuls as pure writers, so each
                        # bank's WAW chain must be rooted in the (drained)
                        # act-init. Only the first matmul takes a semaphore;
                        # the other banks' roots order behind it for free
                        # (PE executes in order).
                        if root_mm is None:
                            root_mm = mm
                            add_dep_helper(
                                mm.ins, guard.ins, sync=True,
                                reason="PSUM must be act-initialized and "
                                       "drained before accumulating",
                            )
                        else:
                            add_dep_helper(
                                mm.ins, root_mm.ins, sync=False,
                                reason="bank root ordered behind guarded "
                                       "root matmul",
                            )
                pss.append(ps)

                if h == 0:
                    # Emit ScalarE's tap-12 product between the half groups:
                    # it fills ScalarE's slot while waiting for the PE to
                    # finish h1 of the previous batch (avoids head-of-line
                    # blocking on the in-order ScalarE queue).
                    prods = {}
                    pt12 = prodp.tile([CPC, L], F16, tag="prod", name=f"sp_{b}_12")
                    nc.scalar.mul(pt12[:], xt[:, 12 : 12 + L], wt[:, 12:13])
                    prods[12] = pt12

            # Tap 10: second half on ScalarE, first half on DVE (keeps
            # ScalarE under the PE's per-batch budget).
            pt10 = prodp.tile([CPC, L], F16, tag="prod", name=f"sp_{b}_10")
            nc.scalar.mul(pt10[:, 2048:L], xt[:, 2048 + 10 : 2048 + 10 + 2048],
                          wt[:, 10:11])

            # previous batch: DVE merges + store. For the last batch, emit
            # this AFTER the fold chain: the epilogue is gated by s(B-1), so
            # the chain must lead the DVE queue there.
            if b > 0 and b < B - 1:
                merge_and_store(b - 1)

            # DVE products + fold chain for this batch (b0 also owns tap 8)
            dve_taps = DVE_TAPS if use_init else DVE_TAPS + [INIT_TAP]
            for k in dve_taps:
                pt = prodp.tile([CPC, L], F16, tag="prod", name=f"dp_{b}_{k}")
                nc.vector.tensor_scalar_mul(pt[:], xt[:, k : k + L], wt[:, k : k + 1])
                prods[k] = pt
            nc.vector.tensor_scalar_mul(pt10[:, 0:2048], xt[:, 10 : 10 + 2048],
                                        wt[:, 10:11])
            prods[10] = pt10
            s = prods[2]
            for i, k in enumerate(dve_taps[1:] + [12, 10]):
                dst = sump.tile([CPC, L], F16, tag="sum", name=f"s_{b}_{i}")
                nc.vector.tensor_tensor(dst[:], prods[k][:], s[:], add)
                s = dst
            if b == B - 1:
                merge_and_store(b - 1)

            acc = accp.tile([CPC, L], F16, tag="acc", name=f"acc_{b}")
            state[b] = {"ps": pss, "s": s, "acc": acc}

        # Epilogue for the last batch: 1024-chunked bridge/merge/store.
        b = B - 1
        acc, s, pss = state[b]["acc"], state[b]["s"], state[b]["ps"]
        # Epilogue stores alternate between the sync and scalar DMA queues:
        # a single queue's ~0.65us issue rate would serialize the tail.
        store_engines = [nc.sync, nc.scalar]
        nst = 0
        ot = op.tile([CPC, L], F16, tag="osb", name=f"o_{b}")
        for c in range(2):
            sl = slice(c * 1024, (c + 1) * 1024)
            nc.scalar.copy(acc[:, sl], pss[0][:, sl])
            nc.vector.tensor_tensor(ot[:, sl], s[:, sl], acc[:, sl], add)
            store_engines[nst % 2].dma_start(out[:, b, sl], ot[:, sl])
            nst += 1
        for c in range(2):
            sl = slice(2048 + c * 512, 2048 + (c + 1) * 512)
            psl = slice(c * 512, (c + 1) * 512)
            nc.scalar.copy(acc[:, sl], pss[1][:, psl])
            nc.vector.tensor_tensor(ot[:, sl], s[:, sl], acc[:, sl], add)
            store_engines[nst % 2].dma_start(out[:, b, sl], ot[:, sl])
            nst += 1
        # Last two chunks: DVE merges straight from PSUM (skips the bridge
        # hop on the serial tail).
        for c in range(2, 4):
            sl = slice(2048 + c * 512, 2048 + (c + 1) * 512)
            psl = slice(c * 512, (c + 1) * 512)
            nc.vector.tensor_tensor(ot[:, sl], pss[1][:, psl], s[:, sl], add)
            store_engines[nst % 2].dma_start(out[:, b, sl], ot[:, sl])
            nst += 1

    nc.compile()
    return nc


def kernel(x: np.ndarray, weight: np.ndarray) -> np.ndarray:
    """x: [8, 4096, 1024] fp32, weight: [15, 1, 1024] fp32 ->
    [8, 4096, 1024] fp32 causal depthwise conv."""
    global _compiled_nc
    if _compiled_nc is None:
        _compiled_nc = _build_nc()
    nc = _compiled_nc

    x = np.ascontiguousarray(x, dtype=np.float32)
    wk = np.ascontiguousarray(weight, dtype=np.float32).reshape(K, D)
    x16 = x.astype(F16NP)
    wk16 = wk.astype(F16NP)

    in_maps = []
    for c in range(NCORES):
        sl = slice(c * CPC, (c + 1) * CPC)
        xpad = np.zeros((CPC, B, LP), dtype=F16NP)
        xpad[:, :, K - 1 :] = x16[:, :, sl].transpose(2, 0, 1)
        dgc = np.zeros((CPC, len(PE_TAPS) * CPC), dtype=F16NP)
        didx = np.arange(CPC)
        for j, k in enumerate(PE_TAPS):
            dgc[didx, j * CPC + didx] = wk16[k, sl]
        wt = np.zeros((CPC, 16), dtype=np.float32)
        wt[:, :K] = wk[:, sl].T
        in_maps.append({"x": xpad, "diag": dgc, "w": wt})

    global _last_in_maps
    _last_in_maps = in_maps
    res = run_bass_kernel_spmd(nc, in_maps, list(range(NCORES)))

    out = np.empty((B, L, D), dtype=np.float32)
    for c in range(NCORES):
        sl = slice(c * CPC, (c + 1) * CPC)
        out[:, :, sl] = res.results[c]["out"].transpose(1, 2, 0).astype(np.float32)
    return out
